# revision 1
# baseline (speedup 1.0000x reference)
"""Trainium2 Bass kernel for nn_AttentionHead (B=2, T=2048, C=2048, H=16 heads, D=128).

Sharding: tensor-parallel over heads — 2 heads per NeuronCore (8 cores).
Each core computes qkv for its heads, RoPE, causal softmax attention, and a
partial c_proj contribution; the host sums the 8 partial outputs.

Layout strategy (per core, heads hA=2c, hB=2c+1):
  - x is pre-transposed on the host to xT [C, B*T] so the contraction dim (C)
    lands on SBUF partitions for the qkv matmuls.
  - q/k weight rows are permuted so RoPE pairs (2i, 2i+1) become (top i, bot
    64+i) within each head, and the qkv matmul emits [Qtop|Ktop] / [Qbot|Kbot]
    psum tiles whose partitions are lane-aligned for the RoPE vector ops.
  - Scores are computed transposed (S.T[k, q]) so the softmax denominator is a
    ones-vector matmul on the PE and P.T feeds the PV matmul directly.
  - All matmuls run in float32r (full PE rate, ~1.5e-4 relative rounding).
"""

import sys

sys.path.insert(0, "/opt/trn_rl_repo")

import numpy as np

import concourse.bass as bass
import concourse.mybir as mybir
import concourse.tile as tile
from concourse import bacc
from concourse.bass_utils import run_bass_kernel_spmd

F32 = mybir.dt.float32
F32R = mybir.dt.float32r

B, T, C, H, D = 2, 2048, 2048, 16, 128
NC_CORES = 8
HPC = H // NC_CORES            # heads per core = 2
BT = B * T                     # 4096
NKT = C // 128                 # 16 contraction tiles
TBS = 512                      # token block size
NTB_B = T // TBS               # 4 token blocks per batch
INV_SQRT_D = 1.0 / float(np.sqrt(D))

_CACHE = {}
_CFG = {"B": B, "T": T, "C": C}


def _build_program():
    B, T, C = _CFG["B"], _CFG["T"], _CFG["C"]
    NKT = C // 128
    NTB_B = T // TBS
    BT = B * T
    nc = bacc.Bacc(None)

    xT = nc.dram_tensor("xT", [C, BT], F32, kind="ExternalInput")
    wqk = nc.dram_tensor("wqk", [128, NKT, 4 * 128], F32, kind="ExternalInput")
    wv = nc.dram_tensor("wv", [128, NKT, HPC * D], F32, kind="ExternalInput")
    wp = nc.dram_tensor("wp", [128, HPC, C], F32, kind="ExternalInput")
    cs = nc.dram_tensor("cs", [128, T], F32, kind="ExternalInput")
    sn = nc.dram_tensor("sn", [128, T], F32, kind="ExternalInput")
    ones_col = nc.dram_tensor("ones_col", [128, 1], F32, kind="ExternalInput")
    ones_row = nc.dram_tensor("ones_row", [1, 128], F32, kind="ExternalInput")
    out_d = nc.dram_tensor("out", [BT, C], F32, kind="ExternalOutput")

    xr = xT[:, :].rearrange("(ko ki) t -> ki ko t", ki=128)

    with tile.TileContext(nc) as tc:
        with (
            tc.tile_pool(name="const", bufs=1) as constp,
            tc.tile_pool(name="xp", bufs=3) as xp,
            tc.tile_pool(name="qk", bufs=1) as qkp,
            tc.tile_pool(name="vp", bufs=1) as vp,
            tc.tile_pool(name="yp", bufs=1) as yp,
            tc.tile_pool(name="pp", bufs=2) as pp,
            tc.tile_pool(name="tmp", bufs=3) as tmpp,
            tc.tile_pool(name="csp", bufs=2) as csp,
            tc.tile_pool(name="rt", bufs=1) as rtp_pool,
            tc.tile_pool(name="bc", bufs=2) as bcp,
            tc.tile_pool(name="rc", bufs=2) as rcp,
            tc.tile_pool(name="outp", bufs=2) as outp,
            tc.tile_pool(name="psA", bufs=3, space="PSUM") as psA,
            tc.tile_pool(name="psB", bufs=2, space="PSUM") as psB,
        ):
            wqk_s = constp.tile([128, NKT, 4 * 128], F32R, tag="wqk")
            wv_s = constp.tile([128, NKT, HPC * D], F32R, tag="wv")
            for k in range(NKT):
                nc.sync.dma_start(wqk_s[:, k, :], wqk[:, k, :].bitcast(F32R))
                nc.sync.dma_start(wv_s[:, k, :], wv[:, k, :].bitcast(F32R))
            wp_s = constp.tile([128, HPC, C], F32R, tag="wp")
            nc.sync.dma_start(wp_s, wp[:, :, :].bitcast(F32R))
            ones_s = constp.tile([128, 1], F32R, tag="onc")
            nc.sync.dma_start(ones_s, ones_col[:, :].bitcast(F32R))
            onesr_s = constp.tile([1, 128], F32R, tag="onr")
            nc.sync.dma_start(onesr_s, ones_row[:, :].bitcast(F32R))

            for b in range(B):
                # ---------------- stage A: qkv + rope -----------------------
                QH = qkp.tile([128, HPC, T], F32R, tag="QH")
                KH = qkp.tile([128, HPC, T], F32R, tag="KH")
                VH = vp.tile([128, NKT, HPC * D], F32R, tag="VH")
                yT = yp.tile([128, HPC, T], F32R, tag="yT")
                for tbl in range(NTB_B):
                    t0 = b * T + tbl * TBS
                    xts = []
                    for k in range(NKT):
                        xt = xp.tile([128, TBS], F32R, tag="x")
                        nc.sync.dma_start(xt, xr[:, k, t0 : t0 + TBS].bitcast(F32R))
                        xts.append(xt)
                    qkT = psA.tile([128, 1024], F32, tag="A")  # [Qtop | Ktop]
                    qkB = psA.tile([128, 1024], F32, tag="A")  # [Qbot | Kbot]
                    vps = psA.tile([128, 1024], F32, tag="A")  # 4 x [tok128, 256]
                    for k in range(NKT):
                        st, sp = (k == 0), (k == NKT - 1)
                        nc.tensor.matmul(qkT[:, 0:512], wqk_s[:, k, 0:128], xts[k], start=st, stop=sp)
                        nc.tensor.matmul(qkB[:, 0:512], wqk_s[:, k, 128:256], xts[k], start=st, stop=sp)
                        nc.tensor.matmul(qkT[:, 512:1024], wqk_s[:, k, 256:384], xts[k], start=st, stop=sp)
                        nc.tensor.matmul(qkB[:, 512:1024], wqk_s[:, k, 384:512], xts[k], start=st, stop=sp)
                        for s in range(4):
                            # vps spans 2 psum banks; start zeroes a whole 2KB
                            # bank region, so emit exactly one start per bank
                            # (s=0 covers s=1's bank, s=2 covers s=3's) and one
                            # stop per bank. Per-element has_written bits make
                            # the first write of each region a plain store.
                            nc.tensor.matmul(
                                vps[:, s * 256 : (s + 1) * 256],
                                xts[k][:, s * 128 : (s + 1) * 128],
                                wv_s[:, k, :],
                                start=(st and s in (0, 2)),
                                stop=(sp and s in (1, 3)),
                            )
                    # rope over [Q|K] jointly: cos/sin broadcast to 1024 wide
                    tcols = slice(tbl * TBS, (tbl + 1) * TBS)
                    cst = csp.tile([128, TBS], F32, tag="cs")
                    nc.sync.dma_start(cst, cs[:, tbl * TBS : (tbl + 1) * TBS])
                    snt = csp.tile([128, TBS], F32, tag="sn")
                    nc.sync.dma_start(snt, sn[:, tbl * TBS : (tbl + 1) * TBS])
                    c_b = cst[:, None, :].broadcast_to([128, 2, TBS])
                    s_b = snt[:, None, :].broadcast_to([128, 2, TBS])
                    t1 = tmpp.tile([128, 2, TBS], F32, tag="t")
                    nc.vector.tensor_mul(t1, qkT.rearrange("p (a n) -> p a n", a=2), c_b)
                    t3 = tmpp.tile([128, 2, TBS], F32, tag="t")
                    nc.vector.tensor_mul(t3, qkT.rearrange("p (a n) -> p a n", a=2), s_b)
                    t2 = tmpp.tile([128, 2, TBS], F32, tag="t")
                    nc.vector.tensor_mul(t2, qkB.rearrange("p (a n) -> p a n", a=2), s_b)
                    rtop = rtp_pool.tile([128, 1024], F32R, tag="rt")
                    nc.vector.tensor_sub(rtop.rearrange("p (a n) -> p a n", a=2), t1, t2)
                    t4 = tmpp.tile([128, 2, TBS], F32, tag="t")
                    nc.vector.tensor_mul(t4, qkB.rearrange("p (a n) -> p a n", a=2), c_b)
                    rbot = rtp_pool.tile([128, 1024], F32R, tag="rt")
                    nc.vector.tensor_add(rbot.rearrange("p (a n) -> p a n", a=2), t3, t4)
                    # regather halves into per-head layout (cross-partition -> DMA)
                    for h in range(HPC):
                        hs = slice(h * 64, (h + 1) * 64)
                        nc.sync.dma_start(QH[0:64, h, tcols], rtop[hs, 0:512])
                        nc.sync.dma_start(QH[64:128, h, tcols], rbot[hs, 0:512])
                        nc.sync.dma_start(KH[0:64, h, tcols], rtop[hs, 512:1024])
                        nc.sync.dma_start(KH[64:128, h, tcols], rbot[hs, 512:1024])
                    # v eviction psum -> sbuf (ACT copy, cast to f32r)
                    for s in range(4):
                        nc.scalar.activation(
                            VH[:, tbl * 4 + s, :],
                            vps[:, s * 256 : (s + 1) * 256],
                            mybir.ActivationFunctionType.Copy,
                        )

                # ---------------- stage B: attention ------------------------
                for h, j in [(hh, jj) for hh in range(HPC) for jj in range(NTB_B)]:
                    n_k = 4 * (j + 1)
                    qsl = slice(j * TBS, (j + 1) * TBS)
                    stripes = []
                    for _si in range((n_k + 7) // 8):
                        p_stripe = pp.tile([128, 4096], F32R, tag="P")
                        stripes.append(p_stripe)

                    def pchunk(m):
                        return stripes[m // 8][:, (m % 8) * 512 : (m % 8) * 512 + 512]

                    for g in range(n_k // 2):
                        sg = psA.tile([128, 1024], F32, tag="A")
                        for u in (0, 1):
                            m = 2 * g + u
                            nc.tensor.matmul(
                                sg[:, u * 512 : (u + 1) * 512],
                                KH[:, h, m * 128 : (m + 1) * 128],
                                QH[:, h, qsl],
                                start=True,
                                stop=True,
                            )
                        dst = stripes[(2 * g) // 8][
                            :, ((2 * g) % 8) * 512 : ((2 * g) % 8) * 512 + 1024
                        ]
                        nc.scalar.activation(
                            dst, sg, mybir.ActivationFunctionType.Exp, scale=INV_SQRT_D
                        )
                        # causal mask on diagonal chunks as soon as exp lands
                        for u in (0, 1):
                            m = 2 * g + u
                            r = m - 4 * j
                            if r >= 0:
                                ck = pchunk(m)
                                nc.gpsimd.affine_select(
                                    out=ck,
                                    in_=ck,
                                    compare_op=mybir.AluOpType.is_ge,
                                    fill=0.0,
                                    base=-(r * 128),
                                    pattern=[[1, 512]],
                                    channel_multiplier=-1,
                                )
                    den = psB.tile([128, 512], F32, tag="B")
                    pv = psB.tile([128, 512], F32, tag="B")
                    for m in range(n_k):
                        nc.tensor.matmul(
                            den[0:1, :], ones_s, pchunk(m),
                            start=(m == 0), stop=(m == n_k - 1),
                        )
                        nc.tensor.matmul(
                            pv, VH[:, m, h * D : (h + 1) * D], pchunk(m),
                            start=(m == 0), stop=(m == n_k - 1),
                        )
                    rc = rcp.tile([1, 512], F32R, tag="rc")
                    with nc.allow_low_precision(
                        reason="f32r is bit-identical storage; recip computes in fp32"
                    ):
                        nc.vector.reciprocal(rc, den[0:1, :])
                    nc.vector.tensor_copy(yT[:, h, qsl], pv)
                    bc = psB.tile([128, 512], F32, tag="B")
                    nc.tensor.matmul(bc, onesr_s, rc, start=True, stop=True)
                    bcs = bcp.tile([128, 512], F32, tag="bc")
                    nc.scalar.activation(bcs, bc, mybir.ActivationFunctionType.Copy)
                    nc.vector.tensor_mul(yT[:, h, qsl], yT[:, h, qsl], bcs)

                # ---------------- stage C: partial c_proj -------------------
                for i in range(T // 128):
                    row0 = b * T + i * 128
                    for n2 in range(C // 1024):
                        ps = psA.tile([128, 1024], F32, tag="A")
                        for n in range(2):
                            col = (n2 * 2 + n) * 512
                            for hh in range(HPC):
                                nc.tensor.matmul(
                                    ps[:, n * 512 : (n + 1) * 512],
                                    yT[:, hh, i * 128 : (i + 1) * 128],
                                    wp_s[:, hh, col : col + 512],
                                    start=(hh == 0),
                                    stop=(hh == HPC - 1),
                                )
                        ot = outp.tile([128, 1024], F32, tag="o")
                        if n2 == 0:
                            nc.scalar.activation(
                                ot, ps, mybir.ActivationFunctionType.Copy
                            )
                        else:
                            nc.vector.tensor_copy(ot, ps)
                        nc.sync.dma_start(
                            out_d[row0 : row0 + 128, n2 * 1024 : (n2 + 1) * 1024], ot
                        )

    nc.compile()
    return nc


def _host_prep(x, w_atten, w_proj):
    """Build the shared + per-core input arrays (all float32, contiguous)."""
    B, T, C = _CFG["B"], _CFG["T"], _CFG["C"]
    NKT = C // 128
    BT = B * T
    x = np.asarray(x, dtype=np.float32)
    w_atten = np.asarray(w_atten, dtype=np.float32)
    w_proj = np.asarray(w_proj, dtype=np.float32)

    xT = np.ascontiguousarray(x.reshape(BT, C).T)  # [C, BT]

    wq = w_atten[0:C]
    wk = w_atten[C : 2 * C]
    wv_full = w_atten[2 * C : 3 * C]

    # rope tables: theta_i = base^(-2i/D)
    theta = 1.0 / (10000.0 ** (np.arange(0, D, 2, dtype=np.float64) / D))  # [64]
    tpos = np.arange(T, dtype=np.float64)
    ang = np.outer(theta, tpos)  # [64, T]
    cs_half = np.cos(ang).astype(np.float32)
    sn_half = np.sin(ang).astype(np.float32)
    cs = np.ascontiguousarray(np.concatenate([cs_half, cs_half], axis=0))  # [128, T]
    sn = np.ascontiguousarray(np.concatenate([sn_half, sn_half], axis=0))

    ones_col = np.ones((128, 1), dtype=np.float32)
    ones_row = np.ones((1, 128), dtype=np.float32)

    top_idx = np.arange(0, D, 2)   # 64
    bot_idx = np.arange(1, D, 2)

    in_maps = []
    for c in range(NC_CORES):
        heads = [HPC * c + h for h in range(HPC)]
        # fb0 (tops of q), fb1 (bots of q), fb2/fb3 same for k
        fb = []
        for wmat in (wq, wk):
            for idx in (top_idx, bot_idx):
                rows = np.concatenate([wmat[hh * D + idx] for hh in heads], axis=0)
                fb.append(rows)  # [128, C]
        w_qk_c = np.concatenate(fb, axis=0)  # [512, C]
        wqk_dev = np.ascontiguousarray(
            w_qk_c.T.reshape(NKT, 128, 4 * 128).transpose(1, 0, 2)
        )
        w_v_c = np.concatenate([wv_full[hh * D : (hh + 1) * D] for hh in heads], axis=0)
        wv_dev = np.ascontiguousarray(
            w_v_c.T.reshape(NKT, 128, HPC * D).transpose(1, 0, 2)
        )
        cols = np.concatenate([np.arange(hh * D, (hh + 1) * D) for hh in heads])
        w_p_c = np.ascontiguousarray(w_proj[:, cols].T)  # [256, C]
        wp_dev = np.ascontiguousarray(
            w_p_c.reshape(HPC, 128, C).transpose(1, 0, 2)
        )
        in_maps.append(
            {
                "xT": xT,
                "wqk": wqk_dev,
                "wv": wv_dev,
                "wp": wp_dev,
                "cs": cs,
                "sn": sn,
                "ones_col": ones_col,
                "ones_row": ones_row,
            }
        )
    return in_maps


def _execute(in_maps, trace=False, trace_kwargs=None):
    if "nc" not in _CACHE:
        _CACHE["nc"] = _build_program()
    nc = _CACHE["nc"]
    kwargs = {}
    if trace:
        _install_ntff_hook()
        kwargs["trace"] = True
        if trace_kwargs:
            kwargs.update(trace_kwargs)
    return run_bass_kernel_spmd(nc, in_maps, core_ids=list(range(NC_CORES)), **kwargs)


def _install_ntff_hook():
    """Restore the axon NTFF profile hook (the container's antenv lacks it)."""
    import types

    if "antenv.axon_hooks" in sys.modules:
        return
    mod = types.ModuleType("antenv.axon_hooks")
    mod._hook = None

    def set_axon_ntff_profile_hook(h):
        mod._hook = h

    def get_axon_ntff_profile_hook():
        if mod._hook is None:
            try:
                from trn_agent_boot.trn_boot import _ntff_profile_via_ctypes

                mod._hook = _ntff_profile_via_ctypes("/opt/axon/libaxon_pjrt.so")
            except Exception:
                mod._hook = None
        return mod._hook

    mod.set_axon_ntff_profile_hook = set_axon_ntff_profile_hook
    mod.get_axon_ntff_profile_hook = get_axon_ntff_profile_hook
    sys.modules["antenv.axon_hooks"] = mod


def kernel(x, w_atten, w_proj):
    in_maps = _host_prep(x, w_atten, w_proj)
    res = _execute(in_maps)
    total = res.results[0]["out"].astype(np.float32)
    for c in range(1, NC_CORES):
        total = total + res.results[c]["out"]
    return total.reshape(B, T, C)



# revision 2
# speedup vs baseline: 1.6970x; 1.6970x over previous
"""Trainium2 Bass kernel for nn_AttentionHead (B=2, T=2048, C=2048, H=16 heads, D=128).

Sharding: tensor-parallel over heads — 2 heads per NeuronCore (8 cores).
Each core computes qkv for its heads, RoPE, causal softmax attention, and a
partial c_proj contribution; the host sums the 8 partial outputs.

v2 layout strategy (per core, heads hA=2c, hB=2c+1):
  - All matmul operands are bf16 (psum accumulation stays f32): halves DMA
    and SBUF traffic, enables fast weight load, same 1 cyc/row PE rate.
  - x is pre-transposed on the host to xT [C, B*T] bf16; one coalesced DMA
    per 512-token block loads all 16 contraction tiles.
  - q/k weight rows are permuted so RoPE pairs (2i, 2i+1) become (top i, bot
    64+i) and the qkv matmul emits [Qtop|Ktop] / [Qbot|Kbot] psum tiles whose
    partitions are lane-aligned for the RoPE vector ops (f32 math, bf16 out).
  - Scores are computed transposed (S.T[k, q]); exp -> bf16 stripes.
  - Softmax denominator via an all-ones 128x128 stationary matmul, which
    replicates the k-sum across all 128 psum partitions; a single fast
    approximate reciprocal + one DVE multiply normalizes the PV output
    (no single-partition reciprocal, no broadcast matmul).
  - c_proj partials evicted as bf16 (ACT/DVE alternating) and written with
    one DMA per 128-token row tile on the scalar HWDGE queue so output
    stores never block input loads on the sync queue.
"""

import sys

sys.path.insert(0, "/opt/trn_rl_repo")

import numpy as np
import ml_dtypes

import concourse.bass as bass
import concourse.mybir as mybir
import concourse.tile as tile
from concourse import bacc
from concourse.bass_utils import run_bass_kernel_spmd

F32 = mybir.dt.float32
BF16 = mybir.dt.bfloat16

B, T, C, H, D = 2, 2048, 2048, 16, 128
NC_CORES = 8
HPC = H // NC_CORES            # heads per core = 2
BT = B * T                     # 4096
NKT = C // 128                 # 16 contraction tiles
TBS = 512                      # token block size
NTB_B = T // TBS               # 4 token blocks per batch
INV_SQRT_D = 1.0 / float(np.sqrt(D))

_CACHE = {}
_CFG = {"B": B, "T": T, "C": C}


def _build_program():
    B, T, C = _CFG["B"], _CFG["T"], _CFG["C"]
    NKT = C // 128
    NTB_B = T // TBS
    BT = B * T
    nc = bacc.Bacc(None)

    xT = nc.dram_tensor("xT", [C, BT], BF16, kind="ExternalInput")
    wqk = nc.dram_tensor("wqk", [128, NKT, 4 * 128], BF16, kind="ExternalInput")
    wv = nc.dram_tensor("wv", [128, NKT, HPC * D], BF16, kind="ExternalInput")
    wp = nc.dram_tensor("wp", [128, HPC, C], BF16, kind="ExternalInput")
    cs = nc.dram_tensor("cs", [128, T], F32, kind="ExternalInput")
    sn = nc.dram_tensor("sn", [128, T], F32, kind="ExternalInput")
    ones_mat = nc.dram_tensor("ones_mat", [128, 128], BF16, kind="ExternalInput")
    out_d = nc.dram_tensor("out", [BT, C], BF16, kind="ExternalOutput")

    xr = xT[:, :].rearrange("(ko ki) t -> ki ko t", ki=128)

    with tile.TileContext(nc) as tc:
        with (
            tc.tile_pool(name="const", bufs=1) as constp,
            tc.tile_pool(name="xp", bufs=2) as xp,
            tc.tile_pool(name="qk", bufs=1) as qkp,
            tc.tile_pool(name="vp", bufs=1) as vp,
            tc.tile_pool(name="yp", bufs=1) as yp,
            tc.tile_pool(name="pp", bufs=2) as pp,
            tc.tile_pool(name="tmp", bufs=3) as tmpp,
            tc.tile_pool(name="rt", bufs=2) as rtp_pool,
            tc.tile_pool(name="rc", bufs=2) as rcp,
            tc.tile_pool(name="outp", bufs=2) as outp,
            tc.tile_pool(name="psA", bufs=3, space="PSUM") as psA,
            tc.tile_pool(name="psB", bufs=2, space="PSUM") as psB,
        ):
            wqk_s = constp.tile([128, NKT, 4 * 128], BF16, tag="wqk")
            nc.sync.dma_start(wqk_s, wqk[:, :, :])
            wv_s = constp.tile([128, NKT, HPC * D], BF16, tag="wv")
            nc.sync.dma_start(wv_s, wv[:, :, :])
            wp_s = constp.tile([128, HPC, C], BF16, tag="wp")
            nc.sync.dma_start(wp_s, wp[:, :, :])
            ones_s = constp.tile([128, 128], BF16, tag="ones")
            nc.sync.dma_start(ones_s, ones_mat[:, :])
            cs_s = constp.tile([128, T], F32, tag="cs")
            nc.sync.dma_start(cs_s, cs[:, :])
            sn_s = constp.tile([128, T], F32, tag="sn")
            nc.sync.dma_start(sn_s, sn[:, :])

            for b in range(B):
                # ---------------- stage A: qkv + rope -----------------------
                QH = qkp.tile([128, HPC, T], BF16, tag="QH")
                KH = qkp.tile([128, HPC, T], BF16, tag="KH")
                VH = vp.tile([128, NKT, HPC * D], BF16, tag="VH")
                yT = yp.tile([128, HPC, T], BF16, tag="yT")
                for tbl in range(NTB_B):
                    t0 = b * T + tbl * TBS
                    xt = xp.tile([128, NKT, TBS], BF16, tag="x")
                    nc.sync.dma_start(xt, xr[:, :, t0 : t0 + TBS])
                    qkT = psA.tile([128, 1024], F32, tag="A")  # [Qtop | Ktop]
                    qkB = psA.tile([128, 1024], F32, tag="A")  # [Qbot | Kbot]
                    vps0 = psB.tile([128, 512], F32, tag="B")  # toks 0-255
                    vps1 = psB.tile([128, 512], F32, tag="B")  # toks 256-511
                    vtiles = (vps0, vps0, vps1, vps1)
                    for k in range(NKT):
                        st, sp = (k == 0), (k == NKT - 1)
                        nc.tensor.matmul(qkT[:, 0:512], wqk_s[:, k, 0:128], xt[:, k, :], start=st, stop=sp)
                        nc.tensor.matmul(qkB[:, 0:512], wqk_s[:, k, 128:256], xt[:, k, :], start=st, stop=sp)
                        nc.tensor.matmul(qkT[:, 512:1024], wqk_s[:, k, 256:384], xt[:, k, :], start=st, stop=sp)
                        nc.tensor.matmul(qkB[:, 512:1024], wqk_s[:, k, 384:512], xt[:, k, :], start=st, stop=sp)
                        for s in range(4):
                            nc.tensor.matmul(
                                vtiles[s][:, (s % 2) * 256 : (s % 2) * 256 + 256],
                                xt[:, k, s * 128 : (s + 1) * 128],
                                wv_s[:, k, :],
                                start=(st and s % 2 == 0),
                                stop=(sp and s % 2 == 1),
                            )
                    # rope over [Q|K] jointly: cos/sin broadcast to 1024 wide
                    tcols = slice(tbl * TBS, (tbl + 1) * TBS)
                    c_b = cs_s[:, None, tcols].broadcast_to([128, 2, TBS])
                    s_b = sn_s[:, None, tcols].broadcast_to([128, 2, TBS])
                    t1 = tmpp.tile([128, 2, TBS], F32, tag="t")
                    nc.vector.tensor_mul(t1, qkT.rearrange("p (a n) -> p a n", a=2), c_b)
                    t3 = tmpp.tile([128, 2, TBS], F32, tag="t")
                    nc.vector.tensor_mul(t3, qkT.rearrange("p (a n) -> p a n", a=2), s_b)
                    t2 = tmpp.tile([128, 2, TBS], F32, tag="t")
                    nc.vector.tensor_mul(t2, qkB.rearrange("p (a n) -> p a n", a=2), s_b)
                    rtop = rtp_pool.tile([128, 2, TBS], BF16, tag="rt")
                    nc.vector.tensor_sub(rtop, t1, t2)
                    t4 = tmpp.tile([128, 2, TBS], F32, tag="t")
                    nc.vector.tensor_mul(t4, qkB.rearrange("p (a n) -> p a n", a=2), c_b)
                    rbot = rtp_pool.tile([128, 2, TBS], BF16, tag="rt")
                    nc.vector.tensor_add(rbot, t3, t4)
                    # regather halves into per-head layout (cross-partition -> DMA)
                    for h in range(HPC):
                        hs = slice(h * 64, (h + 1) * 64)
                        nc.sync.dma_start(QH[0:64, h, tcols], rtop[hs, 0, :])
                        nc.sync.dma_start(QH[64:128, h, tcols], rbot[hs, 0, :])
                        nc.sync.dma_start(KH[0:64, h, tcols], rtop[hs, 1, :])
                        nc.sync.dma_start(KH[64:128, h, tcols], rbot[hs, 1, :])
                    # v eviction psum -> sbuf (ACT copy, cast to bf16)
                    for s in range(4):
                        nc.scalar.activation(
                            VH[:, tbl * 4 + s, :],
                            vtiles[s][:, (s % 2) * 256 : (s % 2) * 256 + 256],
                            mybir.ActivationFunctionType.Copy,
                        )

                # ---------------- stage B: attention ------------------------
                for h, j in [(hh, jj) for hh in range(HPC) for jj in range(NTB_B)]:
                    n_k = 4 * (j + 1)
                    qsl = slice(j * TBS, (j + 1) * TBS)
                    p_stripe = pp.tile([128, 16 * 512], BF16, tag="P")

                    def pchunk(m):
                        return p_stripe[:, m * 512 : (m + 1) * 512]

                    for g in range(n_k // 2):
                        sg = psA.tile([128, 1024], F32, tag="A")
                        for u in (0, 1):
                            m = 2 * g + u
                            nc.tensor.matmul(
                                sg[:, u * 512 : (u + 1) * 512],
                                KH[:, h, m * 128 : (m + 1) * 128],
                                QH[:, h, qsl],
                                start=True,
                                stop=True,
                            )
                        dst = p_stripe[:, (2 * g) * 512 : (2 * g) * 512 + 1024]
                        nc.scalar.activation(
                            dst, sg, mybir.ActivationFunctionType.Exp, scale=INV_SQRT_D
                        )
                        # causal mask on diagonal chunks as soon as exp lands
                        for u in (0, 1):
                            m = 2 * g + u
                            r = m - 4 * j
                            if r >= 0:
                                ck = pchunk(m)
                                nc.gpsimd.affine_select(
                                    out=ck,
                                    in_=ck,
                                    compare_op=mybir.AluOpType.is_ge,
                                    fill=0.0,
                                    base=-(r * 128),
                                    pattern=[[1, 512]],
                                    channel_multiplier=-1,
                                )
                    den = psB.tile([128, 512], F32, tag="B")
                    pv = psB.tile([128, 512], F32, tag="B")
                    for m in range(n_k):
                        nc.tensor.matmul(
                            den, ones_s, pchunk(m),
                            start=(m == 0), stop=(m == n_k - 1),
                        )
                        nc.tensor.matmul(
                            pv, VH[:, m, h * D : (h + 1) * D], pchunk(m),
                            start=(m == 0), stop=(m == n_k - 1),
                        )
                    rden = rcp.tile([128, 512], F32, tag="rc")
                    nc.vector.reciprocal_approx_fast(out=rden, in_=den)
                    nc.vector.tensor_mul(yT[:, h, qsl], pv, rden)

                # ---------------- stage C: partial c_proj -------------------
                for i in range(T // 128):
                    row0 = b * T + i * 128
                    ot = outp.tile([128, C], BF16, tag="o")
                    for n2 in range(C // 1024):
                        ps = psA.tile([128, 1024], F32, tag="A")
                        for n in range(2):
                            col = (n2 * 2 + n) * 512
                            for hh in range(HPC):
                                nc.tensor.matmul(
                                    ps[:, n * 512 : (n + 1) * 512],
                                    yT[:, hh, i * 128 : (i + 1) * 128],
                                    wp_s[:, hh, col : col + 512],
                                    start=(hh == 0),
                                    stop=(hh == HPC - 1),
                                )
                        if n2 == 0:
                            nc.scalar.activation(
                                ot[:, 0:1024], ps, mybir.ActivationFunctionType.Copy
                            )
                        else:
                            nc.vector.tensor_copy(ot[:, 1024:2048], ps)
                    nc.scalar.dma_start(out_d[row0 : row0 + 128, :], ot)

    nc.compile()
    return nc


def _host_prep(x, w_atten, w_proj):
    """Build the shared + per-core input arrays."""
    B, T, C = _CFG["B"], _CFG["T"], _CFG["C"]
    NKT = C // 128
    BT = B * T
    x = np.asarray(x, dtype=np.float32)
    w_atten = np.asarray(w_atten, dtype=np.float32)
    w_proj = np.asarray(w_proj, dtype=np.float32)

    xT = np.ascontiguousarray(x.reshape(BT, C).T.astype(ml_dtypes.bfloat16))

    wq = w_atten[0:C]
    wk = w_atten[C : 2 * C]
    wv_full = w_atten[2 * C : 3 * C]

    # rope tables: theta_i = base^(-2i/D)
    theta = 1.0 / (10000.0 ** (np.arange(0, D, 2, dtype=np.float64) / D))  # [64]
    tpos = np.arange(T, dtype=np.float64)
    ang = np.outer(theta, tpos)  # [64, T]
    cs_half = np.cos(ang).astype(np.float32)
    sn_half = np.sin(ang).astype(np.float32)
    cs = np.ascontiguousarray(np.concatenate([cs_half, cs_half], axis=0))  # [128, T]
    sn = np.ascontiguousarray(np.concatenate([sn_half, sn_half], axis=0))

    ones_mat = np.ones((128, 128), dtype=ml_dtypes.bfloat16)

    top_idx = np.arange(0, D, 2)   # 64
    bot_idx = np.arange(1, D, 2)

    in_maps = []
    for c in range(NC_CORES):
        heads = [HPC * c + h for h in range(HPC)]
        # fb0 (tops of q), fb1 (bots of q), fb2/fb3 same for k
        fb = []
        for wmat in (wq, wk):
            for idx in (top_idx, bot_idx):
                rows = np.concatenate([wmat[hh * D + idx] for hh in heads], axis=0)
                fb.append(rows)  # [128, C]
        w_qk_c = np.concatenate(fb, axis=0)  # [512, C]
        wqk_dev = np.ascontiguousarray(
            w_qk_c.T.reshape(NKT, 128, 4 * 128).transpose(1, 0, 2)
        ).astype(ml_dtypes.bfloat16)
        w_v_c = np.concatenate([wv_full[hh * D : (hh + 1) * D] for hh in heads], axis=0)
        wv_dev = np.ascontiguousarray(
            w_v_c.T.reshape(NKT, 128, HPC * D).transpose(1, 0, 2)
        ).astype(ml_dtypes.bfloat16)
        cols = np.concatenate([np.arange(hh * D, (hh + 1) * D) for hh in heads])
        w_p_c = np.ascontiguousarray(w_proj[:, cols].T)  # [256, C]
        wp_dev = np.ascontiguousarray(
            w_p_c.reshape(HPC, 128, C).transpose(1, 0, 2)
        ).astype(ml_dtypes.bfloat16)
        in_maps.append(
            {
                "xT": xT,
                "wqk": wqk_dev,
                "wv": wv_dev,
                "wp": wp_dev,
                "cs": cs,
                "sn": sn,
                "ones_mat": ones_mat,
            }
        )
    return in_maps


def _execute(in_maps, trace=False, trace_kwargs=None):
    if "nc" not in _CACHE:
        _CACHE["nc"] = _build_program()
    nc = _CACHE["nc"]
    kwargs = {}
    if trace:
        _install_ntff_hook()
        kwargs["trace"] = True
        if trace_kwargs:
            kwargs.update(trace_kwargs)
    return run_bass_kernel_spmd(nc, in_maps, core_ids=list(range(NC_CORES)), **kwargs)


def _install_ntff_hook():
    """Restore the axon NTFF profile hook (the container's antenv lacks it)."""
    import types

    if "antenv.axon_hooks" in sys.modules:
        return
    mod = types.ModuleType("antenv.axon_hooks")
    mod._hook = None

    def set_axon_ntff_profile_hook(h):
        mod._hook = h

    def get_axon_ntff_profile_hook():
        if mod._hook is None:
            try:
                from trn_agent_boot.trn_boot import _ntff_profile_via_ctypes

                mod._hook = _ntff_profile_via_ctypes("/opt/axon/libaxon_pjrt.so")
            except Exception:
                mod._hook = None
        return mod._hook

    mod.set_axon_ntff_profile_hook = set_axon_ntff_profile_hook
    mod.get_axon_ntff_profile_hook = get_axon_ntff_profile_hook
    sys.modules["antenv.axon_hooks"] = mod


def kernel(x, w_atten, w_proj):
    in_maps = _host_prep(x, w_atten, w_proj)
    res = _execute(in_maps)
    total = res.results[0]["out"].astype(np.float32)
    for c in range(1, NC_CORES):
        total = total + res.results[c]["out"].astype(np.float32)
    return total.reshape(B, T, C)
